# revision 19
# baseline (speedup 1.0000x reference)
"""Trainium2 Bass kernel for multi-head attention (B=4, N=2048, C=512, 8 heads).

Sharding: 8 cores = (batch b = core//2) x (head-group g = core%2, 4 heads each).
Per core, a transposed-scores attention pipeline:
  - host supplies x[b] transposed (xT [C, N]) and per-group transposed weights,
    all pre-cast to fp16 (matmul streams at 1 cycle/row; ~4x the mantissa of
    bf16; every tensor here fits fp16 range comfortably)
  - qT/kT stored zero-padded per head ([:, hh, :] has head hh's 64 dims on
    its own partition range, rest zero) so score matmuls contract over the
    full K=128 partition range: same N cycles as K=64, but the PE activity
    monitor sees a fully-active array and keeps the 2.4 GHz clock (K=64
    matmuls measure at the 1.2 GHz throttled rate)
  - v as [N, (1+64) per head] tiles; the leading ones column makes attn@v
    emit the softmax denominator into PSUM partition 0
  - 4 sections (pair x q-half) of 16 ktok blocks; scores^T [128, 1024] PSUM,
    exp on ACT (PSUM -> SBUF fp16), attn@v accumulated in PSUM; attn@v is
    emitted one block behind scores (the PE is in-order - it must never sit
    at an attn@v waiting on a just-issued exp)
  - normalization entirely off the PE: DVE reciprocal (partition 0), GpSimd
    partition_broadcast, DVE multiply, DMA partition-shift into outT
  - output projection on-device; host sums the two half-head partials
"""

import sys

sys.path.insert(0, "/opt/trn_rl_repo")

import numpy as np

B, N, C = 4, 2048, 512
H, D = 8, 64
SCALE = float(D) ** -0.5  # 0.125, exact in fp32
P = 128
CT = C // P  # 4 contraction tiles over channels
NT = N // P  # 16 token blocks
NCORES = 8
FD = 1024  # softmax block free dim (q chunk)
QH = N // FD  # 2 q halves

_cache = {}


def _build():
    import concourse.bacc as bacc
    import concourse.tile as tile
    from concourse import mybir

    f32 = mybir.dt.float32
    f16 = mybir.dt.float16
    u16 = mybir.dt.uint16
    EXP = mybir.ActivationFunctionType.Exp

    nc = bacc.Bacc("TRN2", target_bir_lowering=False, debug=False,
                   num_devices=NCORES)

    xT_d = nc.dram_tensor("xT", [C, N], f16, kind="ExternalInput")
    wqT_d = nc.dram_tensor("wqT", [C, 256], f16, kind="ExternalInput")
    wkT_d = nc.dram_tensor("wkT", [C, 256], f16, kind="ExternalInput")
    wvT_d = nc.dram_tensor("wvT", [C, 256], f16, kind="ExternalInput")
    pwT_d = nc.dram_tensor("pwT", [256, C], f16, kind="ExternalInput")
    zeros_d = nc.dram_tensor("zeros", [64, N], f16, kind="ExternalInput")
    y_d = nc.dram_tensor("y", [N, C], f32, kind="ExternalOutput")

    with tile.TileContext(nc) as tc:
        with (
            tc.tile_pool(name="io", bufs=1) as io,
            tc.tile_pool(name="qk", bufs=1) as qk,
            tc.tile_pool(name="expp", bufs=6) as expp,
            tc.tile_pool(name="workp", bufs=2) as workp,
            tc.tile_pool(name="yp", bufs=3) as yp,
            tc.tile_pool(name="ps_s", bufs=2, space="PSUM") as ps_s,
            tc.tile_pool(name="ps_o", bufs=2, space="PSUM") as ps_o,
        ):
            # ---- input loads ----
            xT_sb = io.tile([P, CT, N], f16, tag="xT", name="xT_sb")
            xT_ap = xT_d[:].rearrange("(t p) n -> p t n", p=P)
            for t in range(CT):
                nc.sync.dma_start(xT_sb[:, t, :], xT_ap[:, t, :])

            wq_sb = io.tile([P, CT, 256], f16, tag="wq", name="wq_sb")
            nc.sync.dma_start(
                wq_sb[:], wqT_d[:].rearrange("(t p) m -> p t m", p=P))
            wk_sb = io.tile([P, CT, 256], f16, tag="wk", name="wk_sb")
            nc.sync.dma_start(
                wk_sb[:], wkT_d[:].rearrange("(t p) m -> p t m", p=P))
            wv_sb = io.tile([P, CT, 256], f16, tag="wv", name="wv_sb")
            nc.sync.dma_start(
                wv_sb[:], wvT_d[:].rearrange("(t p) m -> p t m", p=P))
            pw_sb = io.tile([P, 2, C], f16, tag="pw", name="pw_sb")
            nc.sync.dma_start(
                pw_sb[:], pwT_d[:].rearrange("(t p) m -> p t m", p=P))

            # ---- phase A ----
            qT = []
            kT = []
            vv = []
            outT = []
            for p in range(2):
                qT.append(qk.tile([P, 2, N], f16, tag=f"qT{p}", name=f"qT{p}"))
                kT.append(qk.tile([P, 2, N], f16, tag=f"kT{p}", name=f"kT{p}"))
                vv.append(qk.tile([P, NT, 130], f16, tag=f"v{p}", name=f"v{p}"))
                outT.append(qk.tile([P, N], f16, tag=f"outT{p}", name=f"outT{p}"))

            for p in range(2):
                # zero padding halves via DMA from a host zeros tensor
                nc.sync.dma_start(qT[p][64:128, 0, :], zeros_d[:])
                nc.sync.dma_start(qT[p][0:64, 1, :], zeros_d[:])
                nc.sync.dma_start(kT[p][64:128, 0, :], zeros_d[:])
                nc.sync.dma_start(kT[p][0:64, 1, :], zeros_d[:])
                # ones columns (fp16 1.0) at the head of each v block (DVE:
                # strided 16-element memsets are fast there, slow on GpSimd)
                nc.vector.memset(vv[p][:, :, 0:1].bitcast(u16), 0x3C00)
                nc.vector.memset(vv[p][:, :, 65:66].bitcast(u16), 0x3C00)

            def emit_qk_chunk(p, w_sb, dst, ch):
                pc = slice(128 * p, 128 * (p + 1))
                cs = slice(512 * ch, 512 * (ch + 1))
                ps = ps_s.tile([P, FD], f32, tag="s",
                               name=f"qkps_{p}_{ch}_{w_sb.tensor.name}")
                for t in range(CT):
                    nc.tensor.matmul(
                        ps[:, :512],
                        lhsT=w_sb[:, t, pc],
                        rhs=xT_sb[:, t, cs],
                        start=(t == 0), stop=(t == CT - 1))
                nc.vector.tensor_copy(dst[0:64, 0, cs], ps[0:64, :512])
                nc.scalar.copy(dst[64:128, 1, cs], ps[64:128, :512])

            def emit_v_tile(tt):
                psv = ps_s.tile([P, FD], f32, tag="s", name=f"vps_{tt}")
                for t in range(CT):
                    nc.tensor.matmul(
                        psv[:, :256],
                        lhsT=xT_sb[:, t, 128 * tt:128 * (tt + 1)],
                        rhs=wv_sb[:, t, 0:256],
                        start=(t == 0), stop=(t == CT - 1))
                for p in range(2):
                    pv = psv[:, 128 * p:128 * (p + 1)].rearrange(
                        "p (two d) -> p two d", two=2)
                    dv = vv[p][:, tt, 0:130].rearrange(
                        "p (two d65) -> p two d65", two=2)[:, :, 1:65]
                    nc.vector.tensor_copy(dv, pv)

            fillers = []

            def pop_fillers(k):
                for _ in range(k):
                    if fillers:
                        fillers.pop(0)()

            def norm_head(p, hh, qh, o):
                qs = slice(FD * qh, FD * (qh + 1))
                r = workp.tile([P, FD], f32, tag="r", name=f"r_{p}_{hh}_{qh}")
                nc.vector.reciprocal(r[0:1, :], o[0:1, :])
                rb = workp.tile([65, FD], f32, tag="rb",
                                name=f"rb_{p}_{hh}_{qh}")
                nc.gpsimd.partition_broadcast(rb[:], r[0:1, :])
                st = workp.tile([65, FD], f16, tag="st",
                                name=f"st_{p}_{hh}_{qh}")
                nc.vector.tensor_mul(st[:], o[:], rb[:])
                nc.sync.dma_start(outT[p][64 * hh:64 * (hh + 1), qs],
                                  st[1:65, :])

            def emit_section(p, hh, qh):
                vs = slice(65 * hh, 65 * (hh + 1))
                o = ps_o.tile([65, FD], f32, tag="o", name=f"o_{p}_{hh}_{qh}")

                def emit_scores_exp(i):
                    ks = slice(128 * i, 128 * (i + 1))
                    s = ps_s.tile([P, FD], f32, tag="s",
                                  name=f"s_{p}_{hh}_{qh}_{i}")
                    for j in range(2):
                        js = slice(512 * j, 512 * (j + 1))
                        qj = slice(FD * qh + 512 * j, FD * qh + 512 * (j + 1))
                        nc.tensor.matmul(
                            s[:, js], lhsT=kT[p][:, hh, ks],
                            rhs=qT[p][:, hh, qj], start=True, stop=True)
                    e = expp.tile([P, FD], f16, tag="exp",
                                  name=f"e_{p}_{hh}_{qh}_{i}")
                    nc.scalar.activation(e[:], s[:], EXP)
                    return e

                def emit_attnv(i, e):
                    for j in range(2):
                        js = slice(512 * j, 512 * (j + 1))
                        nc.tensor.matmul(
                            o[:, js], lhsT=vv[p][:, i, vs], rhs=e[:, js],
                            start=(i == 0), stop=(i == NT - 1))

                pending = None
                for i in range(NT):
                    e = emit_scores_exp(i)
                    if pending is not None:
                        emit_attnv(i - 1, pending)
                    pending = e
                    pop_fillers(2 if (p, hh, qh) == (0, 0, 0) else 1)
                emit_attnv(NT - 1, pending)

                norm_head(p, hh, qh, o)

            # critical prefix: everything section (0,0,0) touches early
            for ch in range(4):
                emit_qk_chunk(0, wk_sb, kT[0], ch)
            for ch in range(2):
                emit_qk_chunk(0, wq_sb, qT[0], ch)
            for tt in range(4):
                emit_v_tile(tt)
            # the rest of phase A trickles in between section blocks
            for tt in range(4, NT):
                fillers.append(lambda tt=tt: emit_v_tile(tt))
            for ch in range(2, 4):
                fillers.append(lambda ch=ch: emit_qk_chunk(0, wq_sb, qT[0], ch))
            for ch in range(4):
                fillers.append(lambda ch=ch: emit_qk_chunk(1, wk_sb, kT[1], ch))
            for ch in range(4):
                fillers.append(lambda ch=ch: emit_qk_chunk(1, wq_sb, qT[1], ch))
            for p in range(2):
                for hh in range(2):
                    for qh in range(QH):
                        emit_section(p, hh, qh)
            pop_fillers(len(fillers))

            # ---- phase C: output projection ----
            for tt in range(NT):
                yps = ps_s.tile([P, FD], f32, tag="s", name=f"yps_{tt}")
                for p in range(2):
                    nc.tensor.matmul(
                        yps[:, :512], lhsT=outT[p][:, 128 * tt:128 * (tt + 1)],
                        rhs=pw_sb[:, p, :], start=(p == 0), stop=(p == 1))
                ys = yp.tile([P, C], f32, tag="y", name=f"ys_{tt}")
                if tt % 2 == 0:
                    nc.scalar.copy(ys[:], yps[:, :512])
                else:
                    nc.vector.tensor_copy(ys[:], yps[:, :512])
                nc.sync.dma_start(y_d[128 * tt:128 * (tt + 1), :], ys[:])

    nc.finalize()
    return nc


def _get_nc():
    if "nc" not in _cache:
        _cache["nc"] = _build()
    return _cache["nc"]


def _make_in_maps(x, q_w, kv_w, proj_w):
    x = np.asarray(x, dtype=np.float32)
    q_w = np.asarray(q_w, dtype=np.float32)
    kv_w = np.asarray(kv_w, dtype=np.float32)
    proj_w = np.asarray(proj_w, dtype=np.float32)
    f16 = np.float16
    in_maps = []
    for core in range(NCORES):
        b, g = core // 2, core % 2
        hs = slice(g * 256, (g + 1) * 256)
        in_maps.append({
            "xT": np.ascontiguousarray(x[b].T.astype(f16)),
            "wqT": np.ascontiguousarray((q_w[hs, :] * np.float32(SCALE)).T
                                        .astype(f16)),
            "wkT": np.ascontiguousarray(kv_w[hs, :].T.astype(f16)),
            "wvT": np.ascontiguousarray(
                kv_w[C + g * 256:C + (g + 1) * 256, :].T.astype(f16)),
            "pwT": np.ascontiguousarray(proj_w[:, hs].T.astype(f16)),
            "zeros": np.zeros((64, N), dtype=f16),
        })
    return in_maps


def kernel(x, q_w, kv_w, proj_w, proj_b, H=None, W=None, _trace=False):
    from concourse.bass_utils import run_bass_kernel_spmd

    nc = _get_nc()
    in_maps = _make_in_maps(x, q_w, kv_w, proj_w)
    res = run_bass_kernel_spmd(nc, in_maps, core_ids=list(range(NCORES)),
                               trace=_trace)
    proj_b = np.asarray(proj_b, dtype=np.float32)
    out = np.empty((B, N, C), dtype=np.float32)
    for b in range(B):
        out[b] = res.results[2 * b]["y"] + res.results[2 * b + 1]["y"] + proj_b
    if _trace:
        return out, res
    return out
